# revision 1
# baseline (speedup 1.0000x reference)
"""Trainium2 Bass kernel for nn_CrossAttention_15006615733765 (raw Bass, no Tile).

Mathematical structure: the reference broadcasts a per-batch context vector
(B, CTX_DIM) to every spatial position before projecting to K/V.  All keys
within a batch are therefore identical, softmax over the key axis is exactly
uniform, and the attention output equals V itself.  The module collapses to

    out[b, c, h, w] = ((context[b] @ Wv) @ Wo + bo)[c]

independent of x, Wq and Wk (exact in infinite precision; measured rel err
vs the fp32 reference is ~4e-7).  The kernel computes the two small matmuls
on the tensor engine and materializes the broadcast output shard per core,
sharding the 512 output channels across 8 cores.

Raw Bass (no Tile) with hand-placed semaphores.  Engine plan:
  Sync   : Wv chunks 0/2/4 (HWDGE queue 1), final broadcast store
  Scalar : ctx + Wv chunks 1/3/5 + consts (HWDGE queue 2)
  Tensor : HAM-warmup -> stage1 matmuls -> transposes -> stage2 -> selectors
  Vector : PSUM -> SBUF copies between PE stages (split to overlap PE work)
  GpSimd : unused (block exits with no_gpsimd_drain)
The store has no explicit completion wait: the block-exit DRAIN on the sync
engine waits for the HWDGE queue, so the ~6.5 us walrus semaphore-reset
epilogue overlaps the output transfer instead of following it.
"""

import numpy as np

import concourse.bacc as bacc
import concourse.mybir as mybir
from concourse.bass_utils import run_bass_kernel_spmd

B, DIM, CTX_DIM = 4, 512, 768
H = W = 48
NPOS = H * W
NCORES = 8
CPC = DIM // NCORES
P = 128
KC = CTX_DIM // P
KD = DIM // P
NREP = NPOS // P
ROW = B * CPC
F32 = mybir.dt.float32

_CACHE: dict = {}


def _build_nc():
    nc = bacc.Bacc("TRN2", target_bir_lowering=False, debug=False, num_devices=NCORES)

    ctxc = nc.dram_tensor("ctxc", [P, KC, B], F32, kind="ExternalInput")
    wvc = nc.dram_tensor("wvc", [P, KC, DIM], F32, kind="ExternalInput")
    woc = nc.dram_tensor("woc", [P, KD, CPC], F32, kind="ExternalInput")
    selc = nc.dram_tensor("selc", [B + 1, B, P], F32, kind="ExternalInput")
    idc = nc.dram_tensor("idc", [B, B], F32, kind="ExternalInput")
    boc = nc.dram_tensor("boc", [1, CPC], F32, kind="ExternalInput")
    outd = nc.dram_tensor("outd", [NPOS, ROW], F32, kind="ExternalOutput")

    ctx_sb = nc.alloc_sbuf_tensor("ctx_sb", [P, KC, B], F32).ap()
    wv_sb = nc.alloc_sbuf_tensor("wv_sb", [P, KC, DIM], F32).ap()
    wo_sb = nc.alloc_sbuf_tensor("wo_sb", [P, KD, CPC], F32).ap()
    sel_sb = nc.alloc_sbuf_tensor("sel_sb", [B + 1, B, P], F32).ap()
    id_sb = nc.alloc_sbuf_tensor("id_sb", [B, B], F32).ap()
    o5_sb = nc.alloc_sbuf_tensor("o5_sb", [B + 1, CPC], F32).ap()
    t_sb = nc.alloc_sbuf_tensor("t_sb", [B, DIM], F32).ap()
    tT_sb = nc.alloc_sbuf_tensor("tT_sb", [P, KD, B], F32).ap()
    # Output row triplicated per partition -> 3 KiB DMA descriptors.
    NDUP = 3
    rep_sb = nc.alloc_sbuf_tensor("rep_sb", [P, NDUP, ROW], F32).ap()

    pt = nc.alloc_psum_tensor("pt", [B, DIM], F32).ap()
    ptT = nc.alloc_psum_tensor("ptT", [P, KD, B], F32).ap()
    po = nc.alloc_psum_tensor("po", [B, CPC], F32).ap()
    prep = nc.alloc_psum_tensor("prep", [P, B, CPC], F32).ap()
    pwarm = nc.alloc_psum_tensor("pwarm", [B, DIM], F32).ap()

    from contextlib import ExitStack

    with ExitStack() as stack:
        s_ctx = stack.enter_context(nc.semaphore("s_ctx"))
        s_wv = [stack.enter_context(nc.semaphore(f"s_wv{k}")) for k in range(KC)]
        s_const = stack.enter_context(nc.semaphore("s_const"))
        s_mm = stack.enter_context(nc.semaphore("s_mm"))
        s_tcp = stack.enter_context(nc.semaphore("s_tcp"))
        s_tTcp = stack.enter_context(nc.semaphore("s_tTcp"))
        s_o5 = stack.enter_context(nc.semaphore("s_o5"))
        s_rep = stack.enter_context(nc.semaphore("s_rep"))
        s_r0 = stack.enter_context(nc.semaphore("s_r0"))
        s_out = stack.enter_context(nc.semaphore("s_out"))

        with nc.Block(no_gpsimd_drain=True) as block:

            @block.sync
            def _(sync):
                # wv0 first and alone on this queue so it gets near-full
                # SDMA bandwidth; the PE's start is gated on it.
                for k in range(2):
                    sync.dma_start(
                        out=wv_sb[:, k, :], in_=wvc[:, k, :]
                    ).then_inc(s_wv[k], 16)
                sync.wait_ge(s_rep, 2)
                # pos = r*384 + p*3 + d: each partition contributes 3072-byte
                # contiguous chunks (three consecutive 256-float rows).
                # Issued from both HWDGE engines (r halves) so issue and
                # descriptor generation run in parallel; both halves span
                # all 128 partitions.
                out_view = outd.rearrange("(r p d) n -> p r (d n)", p=P, d=3)
                src_view = (
                    rep_sb.rearrange("p d n -> p (d n)")[:, None, :]
                    .broadcast_to((P, NPOS // (3 * P), 3 * ROW))
                )
                # No explicit completion wait: the block-exit DRAIN on each
                # issuing engine waits for its HWDGE queue to empty, so the
                # barrier and walrus sem-reset epilogue overlap the transfer.
                sync.dma_start(out=out_view, in_=src_view).then_inc(s_out, 16)

            @block.scalar
            def _(scalar):
                scalar.dma_start(out=ctx_sb[:], in_=ctxc[:]).then_inc(s_ctx, 16)
                for k in range(2, KC):
                    scalar.dma_start(
                        out=wv_sb[:, k, :], in_=wvc[:, k, :]
                    ).then_inc(s_wv[k], 16)
                scalar.dma_start(out=wo_sb[:], in_=woc[:]).then_inc(s_const, 16)
                scalar.dma_start(out=sel_sb[:], in_=selc[:]).then_inc(s_const, 16)
                scalar.dma_start(out=id_sb[:], in_=idc[:]).then_inc(s_const, 16)
                scalar.dma_start(
                    out=o5_sb[B:B + 1, :], in_=boc[:]
                ).then_inc(s_const, 16)


            @block.tensor
            def _(tensor):
                # HAM warmup: ungated dummy matmuls (garbage SBUF data,
                # scratch PSUM) keep the PE busy from boot until the first
                # Wv chunk lands, ramping the PE clock from 1.2 to 2.4 GHz.
                NWARM = 2
                for _w in range(NWARM):
                    nc.tensor.matmul(
                        pwarm[:],
                        ctx_sb[:, 0, :],
                        wv_sb[:, KC - 1, :],
                        start=(_w == 0),
                        stop=(_w == NWARM - 1),
                    )
                tensor.wait_ge(s_ctx, 16)
                for k in range(KC):
                    tensor.wait_ge(s_wv[k], 16)
                    ins = nc.tensor.matmul(
                        pt[:],
                        ctx_sb[:, k, :],
                        wv_sb[:, k, :],
                        start=(k == 0),
                        stop=(k == KC - 1),
                    )
                ins.then_inc(s_mm, 1)

                tensor.wait_ge(s_tcp, 1)
                tensor.wait_ge(s_const, 64)
                for m in range(KD // 2):
                    ins = nc.tensor.transpose(
                        ptT[:, m, :], t_sb[:, m * P:(m + 1) * P], id_sb[:]
                    )
                ins.then_inc(s_mm, 1)
                tensor.wait_ge(s_tcp, 2)
                for m in range(KD // 2, KD):
                    ins = nc.tensor.transpose(
                        ptT[:, m, :], t_sb[:, m * P:(m + 1) * P], id_sb[:]
                    )
                ins.then_inc(s_mm, 1)

                # stage2 m=0,1 gated on the first tT half-copy, m=2,3 on the
                # second; the DVE copies overlap the remaining transposes.
                tensor.wait_ge(s_tTcp, 1)
                for m in range(KD // 2):
                    nc.tensor.matmul(
                        po[:],
                        tT_sb[:, m, :],
                        wo_sb[:, m, :],
                        start=(m == 0),
                        stop=False,
                    )
                tensor.wait_ge(s_tTcp, 2)
                for m in range(KD // 2, KD):
                    ins = nc.tensor.matmul(
                        po[:],
                        tT_sb[:, m, :],
                        wo_sb[:, m, :],
                        start=False,
                        stop=(m == KD - 1),
                    )
                ins.then_inc(s_mm, 1)

                tensor.wait_ge(s_o5, 1)
                for b in range(B):
                    ins = nc.tensor.matmul(
                        prep[:, b, :],
                        sel_sb[:, b, :],
                        o5_sb[:, :],
                        start=True,
                        stop=True,
                    )
                ins.then_inc(s_mm, 1)

            @block.vector
            def _(vector):
                vector.wait_ge(s_mm, 1)
                nc.vector.tensor_copy(
                    t_sb[:, :DIM // 2], pt[:, :DIM // 2]
                ).then_inc(s_tcp, 1)
                nc.vector.tensor_copy(
                    t_sb[:, DIM // 2:], pt[:, DIM // 2:]
                ).then_inc(s_tcp, 1)
                vector.wait_ge(s_mm, 2)
                nc.vector.tensor_copy(
                    tT_sb[:, :KD // 2, :], ptT[:, :KD // 2, :]
                ).then_inc(s_tTcp, 1)
                vector.wait_ge(s_mm, 3)
                nc.vector.tensor_copy(
                    tT_sb[:, KD // 2:, :], ptT[:, KD // 2:, :]
                ).then_inc(s_tTcp, 1)
                vector.wait_ge(s_mm, 4)
                nc.vector.tensor_copy(o5_sb[:B, :], po[:]).then_inc(s_o5, 1)
                vector.wait_ge(s_mm, 5)
                flat = prep[:].rearrange("p b c -> p (b c)")
                nc.vector.tensor_copy(rep_sb[:, 0, :], flat)
                # replicas 1-2 in one SBUF->SBUF copy (broadcast source)
                nc.vector.tensor_copy(
                    rep_sb[:, 1:, :],
                    rep_sb[:, 0:1, :].broadcast_to((P, NDUP - 1, ROW)),
                ).then_inc(s_rep, 2)

    nc.compile()
    return nc


def _get_nc():
    if "nc" not in _CACHE:
        _CACHE["nc"] = _build_nc()
    return _CACHE["nc"]


def _prepare_in_maps(context, Wv, Wo, bo):
    context = np.ascontiguousarray(context, dtype=np.float32)
    Wv = np.ascontiguousarray(Wv, dtype=np.float32)
    Wo = np.ascontiguousarray(Wo, dtype=np.float32)
    bo = np.ascontiguousarray(bo, dtype=np.float32)

    ctxc = np.ascontiguousarray(context.T.reshape(KC, P, B).transpose(1, 0, 2))
    wvc = np.ascontiguousarray(Wv.reshape(KC, P, DIM).transpose(1, 0, 2))
    wo_chunk = Wo.reshape(KD, P, DIM).transpose(1, 0, 2)

    selc = np.zeros((B + 1, B, P), dtype=np.float32)
    for b in range(B):
        selc[b, b, :] = 1.0
        selc[B, b, :] = 1.0
    idc = np.eye(B, dtype=np.float32)

    in_maps = []
    for i in range(NCORES):
        woc = np.ascontiguousarray(wo_chunk[:, :, i * CPC:(i + 1) * CPC])
        boc = np.ascontiguousarray(bo[i * CPC:(i + 1) * CPC]).reshape(1, CPC)
        in_maps.append(
            {
                "ctxc": ctxc,
                "wvc": wvc,
                "woc": woc,
                "selc": selc,
                "idc": idc,
                "boc": boc,
            }
        )
    return in_maps


def _unshard(results):
    shards = np.stack([r["outd"] for r in results], axis=0)
    shards = shards.reshape(NCORES, NPOS, B, CPC)
    out = shards.transpose(2, 0, 3, 1).reshape(B, DIM, H, W)
    return np.ascontiguousarray(out)


def kernel(x, context, Wq, Wk, Wv, Wo, bo):
    del x, Wq, Wk
    nc = _get_nc()
    in_maps = _prepare_in_maps(context, Wv, Wo, bo)
    results = run_bass_kernel_spmd(nc, in_maps, list(range(NCORES))).results
    return _unshard(results)



# revision 11
# speedup vs baseline: 1.0891x; 1.0891x over previous
"""Trainium2 Bass kernel for nn_CrossAttention_15006615733765 (raw Bass, no Tile).

Mathematical structure: the reference broadcasts a per-batch context vector
(B, CTX_DIM) to every spatial position before projecting to K/V.  All keys
within a batch are therefore identical, softmax over the key axis is exactly
uniform, and the attention output equals V itself.  The module collapses to

    out[b, c, h, w] = ((context[b] @ Wv) @ Wo + bo)[c]

independent of x, Wq and Wk.  The kernel computes the two small matmuls on
the tensor engine in bf16 (tolerance 2e-2 >> bf16 rounding ~1e-3) and
materializes the broadcast output shard per core, sharding the 512 output
channels across 8 cores.

v2 design (from trace analysis of the f32 v1 at 23.4us):
  - measured exec window ends ~7us after the kernel body exits (walrus
    resets all 254 semaphores, Tensor's 52 resets at ~115ns are the long
    pole), concurrently with the 2.36MB output store (~7us at ~337GB/s).
    Both tails are ~equal, so exec ~= y_ready + 2.7us.  Everything below
    minimizes y_ready.
  - bf16 weights halve the input DMA (1.65MB -> 0.86MB) and halve the PE
    passes (no fp32 LOW/HIGH double pumping).
  - ctx chunks are packed WITH the Wv chunks (wvx[:, k, 0:4] = ctx chunk)
    so one dma_start per chunk-group feeds both operands; DMA_DIRECT2D
    issue costs ~0.65us each, so chunk DMAs are batched 1+2 per queue.
  - stage1 is 2-way column-tiled on the PE (tile (0,0) streams wv cols
    0:256 into psum rows 0:4, tile (0,32) streams cols 256:512 into psum
    rows 32:36) so the two halves run concurrently in the array.
  - the final broadcast is ONE matmul (ones [5,128] stationary x block-diag
    o5big [5,256]) instead of 4 selector matmuls; the block-diag is built
    by a single DVE masked multiply from po.
  - bias enters as a host-prepared row of the const tensor (o5big row 4).
  - the output store is split across both HWDGE queues (sync/scalar).
Engine plan:
  Sync   : wvx chunks 0,1-2; output store half A
  Scalar : wvx chunks 3,4-5; rep replica 1 copy; output store half B
  GpSimd : consts (SWDGE), otherwise idle (block exits no_gpsimd_drain)
  Tensor : warmup -> stage1 (col-tiled) -> transposes -> stage2 -> bcast
  Vector : PSUM->SBUF copies, masked multiply, rep replicas 0/2
"""

import numpy as np
import ml_dtypes

import concourse.bacc as bacc
import concourse.mybir as mybir
from concourse.bass_utils import run_bass_kernel_spmd

B, DIM, CTX_DIM = 4, 512, 768
H = W = 48
NPOS = H * W
NCORES = 8
CPC = DIM // NCORES          # 64 channels per core
P = 128
KC = CTX_DIM // P            # 6 contraction chunks
KD = DIM // P                # 4 d-chunks
ROW = B * CPC                # 256 floats per output row
NDUP = 3                     # row triplication -> 3 KiB store descriptors
F32 = mybir.dt.float32
BF16 = mybir.dt.bfloat16
BFNP = ml_dtypes.bfloat16

# consts2 [5, 648] column layout
C_ONES = 0        # [5, 128]  all-ones selector (stationary of bcast matmul)
C_MASK = 128      # [4, 256]  block-diag mask
C_O5 = 384        # [5, 256]  o5big: rows 0-3 runtime (masked y), row 4 bias
C_ID = 640        # [4, 8]    f32 4x4 identity, stored as bf16 bit pattern
C2W = 648

_CACHE: dict = {}
NWARM = 3  # ungated PE warmup matmuls (0 for CoreSim: it rejects
           # reads of uninitialized SBUF)
COLTILE = False  # 2-way PE column tiling for stage1


def _build_nc():
    nc = bacc.Bacc("TRN2", target_bir_lowering=False, debug=False, num_devices=NCORES)

    wvx = nc.dram_tensor("wvx", [P, KC, B + DIM], BF16, kind="ExternalInput")
    constw = nc.dram_tensor("constw", [P, KD * CPC], BF16, kind="ExternalInput")
    consts2 = nc.dram_tensor("consts2", [36, C2W], BF16, kind="ExternalInput")
    idc = nc.dram_tensor("idc", [B, B], F32, kind="ExternalInput")
    outd = nc.dram_tensor("outd", [NPOS, ROW], F32, kind="ExternalOutput")

    wvx_sb = nc.alloc_sbuf_tensor("wvx_sb", [P, KC, B + DIM], BF16).ap()
    constw_sb = nc.alloc_sbuf_tensor("constw_sb", [P, KD * CPC], BF16).ap()
    consts2_sb = nc.alloc_sbuf_tensor("consts2_sb", [36, C2W], BF16).ap()
    idc_sb = nc.alloc_sbuf_tensor("idc_sb", [36, B], F32).ap()
    t_sb = nc.alloc_sbuf_tensor("t_sb", [36, DIM // 2], F32).ap()
    tf_sb = nc.alloc_sbuf_tensor("tf_sb", [B, DIM], F32).ap()
    tT_sb = nc.alloc_sbuf_tensor("tT_sb", [P, KD, B], BF16).ap()
    po_sb = nc.alloc_sbuf_tensor("po_sb", [B, CPC], BF16).ap()
    rep_sb = nc.alloc_sbuf_tensor("rep_sb", [P, NDUP, ROW], F32).ap()

    pt_a = nc.alloc_psum_tensor("pt_a", [B, DIM // 2], F32).ap()
    pt_b = nc.alloc_psum_tensor("pt_b", [36, DIM // 2], F32).ap()
    pwide = nc.alloc_psum_tensor("pwide", [B, DIM], F32).ap()
    ptT = nc.alloc_psum_tensor("ptT", [P, KD, B], F32).ap()
    po = nc.alloc_psum_tensor("po", [B, CPC], F32).ap()
    prep = nc.alloc_psum_tensor("prep", [P, ROW], F32).ap()
    pwarm = nc.alloc_psum_tensor("pwarm", [B, DIM], F32).ap()

    id_f32 = idc_sb[0:B, :]
    id_f32_hi = idc_sb[32:36, :]

    from contextlib import ExitStack

    with ExitStack() as stack:
        s_w0 = stack.enter_context(nc.semaphore("s_w0"))
        s_w12 = stack.enter_context(nc.semaphore("s_w12"))
        s_w3 = stack.enter_context(nc.semaphore("s_w3"))
        s_w45 = stack.enter_context(nc.semaphore("s_w45"))
        s_const = stack.enter_context(nc.semaphore("s_const"))
        s_mm1 = stack.enter_context(nc.semaphore("s_mm1"))
        s_tcp = stack.enter_context(nc.semaphore("s_tcp"))
        s_mm2 = stack.enter_context(nc.semaphore("s_mm2"))
        s_tTcp = stack.enter_context(nc.semaphore("s_tTcp"))
        s_mm3 = stack.enter_context(nc.semaphore("s_mm3"))
        s_o5 = stack.enter_context(nc.semaphore("s_o5"))
        s_po = stack.enter_context(nc.semaphore("s_po"))
        s_mm4 = stack.enter_context(nc.semaphore("s_mm4"))
        s_rep = stack.enter_context(nc.semaphore("s_rep"))
        s_out = stack.enter_context(nc.semaphore("s_out"))

        out_view = outd.rearrange("(r p d) n -> p r (d n)", p=P, d=NDUP)
        src_view = (
            rep_sb.rearrange("p d n -> p (d n)")[:, None, :]
            .broadcast_to((P, NPOS // (NDUP * P), NDUP * ROW))
        )
        RHALF = NPOS // (NDUP * P) // 2  # 3

        with nc.Block(no_gpsimd_drain=True) as block:

            @block.sync
            def _(sync):
                sync.dma_start(
                    out=wvx_sb[:, 0, :], in_=wvx[:, 0, :]
                ).then_inc(s_w0, 16)
                sync.dma_start(
                    out=wvx_sb[:, 1:3, :], in_=wvx[:, 1:3, :]
                ).then_inc(s_w12, 16)
                sync.wait_ge(s_rep, 3)
                sync.dma_start(
                    out=out_view[:, 0:RHALF, :], in_=src_view[:, 0:RHALF, :]
                ).then_inc(s_out, 16)

            @block.scalar
            def _(scalar):
                scalar.dma_start(
                    out=wvx_sb[:, 3, :], in_=wvx[:, 3, :]
                ).then_inc(s_w3, 16)
                scalar.dma_start(
                    out=wvx_sb[:, 4:6, :], in_=wvx[:, 4:6, :]
                ).then_inc(s_w45, 16)
                scalar.dma_start(out=consts2_sb[:], in_=consts2[:]).then_inc(
                    s_const, 16
                )
                scalar.dma_start(out=constw_sb[:], in_=constw[:]).then_inc(
                    s_const, 16
                )
                scalar.dma_start(
                    out=idc_sb[0:B, :], in_=idc[:]
                ).then_inc(s_const, 16)
                scalar.dma_start(
                    out=idc_sb[32:36, :], in_=idc[:]
                ).then_inc(s_const, 16)
                scalar.wait_ge(s_rep, 3)
                scalar.dma_start(
                    out=out_view[:, RHALF:, :], in_=src_view[:, RHALF:, :]
                ).then_inc(s_out, 16)

            @block.gpsimd
            def _(gpsimd):
                pass

            @block.tensor
            def _(tensor):
                # Ungated warmup matmuls on garbage SBUF ramp the PE clock
                # (1.2 -> 2.4 GHz needs ~4-5us of sustained activity).
                for w in range(NWARM):
                    nc.tensor.matmul(
                        pwarm[:],
                        wvx_sb[:, 0, 0:B],
                        wvx_sb[:, KC - 1, B:],
                        start=(w == 0),
                        stop=(w == NWARM - 1),
                    )

                # stage1: t[b, d] = sum_c ctx[b, c] Wv[c, d]
                HN = DIM // 2
                order = [(0, s_w0), (3, s_w3), (1, s_w12), (2, None),
                         (4, s_w45), (5, None)]
                seen = 0
                if COLTILE:
                    # Tile A (array cols 0-3) streams wv cols 0:256 ->
                    # pt_a[0:4], tile B (cols 32-35) streams cols 256:512
                    # -> pt_b[32:36]; the two halves run concurrently.
                    last_a = last_b = None
                    for k, sem in order:
                        if sem is not None:
                            tensor.wait_ge(sem, 16)
                        last_a = nc.tensor.matmul(
                            pt_a[:],
                            wvx_sb[:, k, 0:B],
                            wvx_sb[:, k, B:B + HN],
                            start=(seen == 0),
                            stop=(seen == KC - 1),
                            tile_position=(0, 0),
                            skip_group_check=True,
                        )
                        last_b = nc.tensor.matmul(
                            pt_b[32:32 + B, :],
                            wvx_sb[:, k, 0:B],
                            wvx_sb[:, k, B + HN:],
                            start=(seen == 0),
                            stop=(seen == KC - 1),
                            tile_position=(0, 32),
                            skip_group_check=True,
                        )
                        seen += 1
                    last_a.then_inc(s_mm1, 1)
                    last_b.then_inc(s_mm1, 1)
                else:
                    ins = None
                    for k, sem in order:
                        if sem is not None:
                            tensor.wait_ge(sem, 16)
                        ins = nc.tensor.matmul(
                            pwide[:],
                            wvx_sb[:, k, 0:B],
                            wvx_sb[:, k, B:],
                            start=(seen == 0),
                            stop=(seen == KC - 1),
                        )
                        seen += 1
                    ins.then_inc(s_mm1, 2)

                # transposes: tT[d, b] per 128-d chunk (f32, identity mult)
                tensor.wait_ge(s_const, 64)
                if COLTILE:
                    tensor.wait_ge(s_tcp, 1)
                    nc.tensor.transpose(ptT[:, 0, :], t_sb[0:B, 0:P], id_f32)
                    ins = nc.tensor.transpose(
                        ptT[:, 1, :], t_sb[0:B, P:2 * P], id_f32
                    )
                    tensor.wait_ge(s_tcp, 2)
                    nc.tensor.transpose(
                        ptT[:, 2, :], t_sb[32:32 + B, 0:P], id_f32_hi
                    )
                    ins = nc.tensor.transpose(
                        ptT[:, 3, :], t_sb[32:32 + B, P:2 * P], id_f32_hi
                    )
                else:
                    tensor.wait_ge(s_tcp, 1)
                    nc.tensor.transpose(ptT[:, 0, :], tf_sb[:, 0:P], id_f32)
                    ins = nc.tensor.transpose(
                        ptT[:, 1, :], tf_sb[:, P:2 * P], id_f32
                    )
                    tensor.wait_ge(s_tcp, 2)
                    nc.tensor.transpose(ptT[:, 2, :], tf_sb[:, 2 * P:3 * P], id_f32)
                    ins = nc.tensor.transpose(
                        ptT[:, 3, :], tf_sb[:, 3 * P:], id_f32
                    )
                ins.then_inc(s_mm2, 1)

                # stage2: po[b, c] = sum_d tT[d, b] Wo[d, c]
                tensor.wait_ge(s_tTcp, 1)
                for m in range(KD):
                    ins = nc.tensor.matmul(
                        po[:],
                        tT_sb[:, m, :],
                        constw_sb[:, m * CPC:(m + 1) * CPC],
                        start=(m == 0),
                        stop=(m == KD - 1),
                    )
                ins.then_inc(s_mm3, 1)

                # broadcast: prep[p, n] = sum_k ones[k] * o5big[k, n]
                #          = y[b(n), c(n)] + bo[c(n)]  on every partition
                tensor.wait_ge(s_o5, 1)
                ins = nc.tensor.matmul(
                    prep[:],
                    consts2_sb[0:5, C_ONES:C_ONES + P],
                    consts2_sb[0:5, C_O5:C_O5 + ROW],
                    start=True,
                    stop=True,
                )
                ins.then_inc(s_mm4, 1)

            @block.vector
            def _(vector):
                HN = DIM // 2
                vector.wait_ge(s_mm1, 2)
                if COLTILE:
                    nc.vector.tensor_copy(
                        t_sb[0:B, :], pt_a[:]
                    ).then_inc(s_tcp, 1)
                    nc.vector.tensor_copy(
                        t_sb[32:32 + B, :], pt_b[32:32 + B, :]
                    ).then_inc(s_tcp, 1)
                else:
                    nc.vector.tensor_copy(
                        tf_sb[:, 0:DIM // 2], pwide[:, 0:DIM // 2]
                    ).then_inc(s_tcp, 1)
                    nc.vector.tensor_copy(
                        tf_sb[:, DIM // 2:], pwide[:, DIM // 2:]
                    ).then_inc(s_tcp, 1)
                vector.wait_ge(s_mm2, 1)
                nc.vector.tensor_copy(tT_sb[:], ptT[:]).then_inc(s_tTcp, 1)
                # masked multiply builds the block-diag o5big rows 0-3
                vector.wait_ge(s_mm3, 1)
                vector.wait_ge(s_const, 64)
                nc.vector.tensor_copy(po_sb[:], po[:]).then_inc(s_po, 1)
                vector.wait_ge(s_po, 1)
                nc.vector.tensor_tensor(
                    consts2_sb[0:B, C_O5:C_O5 + ROW].rearrange(
                        "p (a c) -> p a c", a=B
                    ),
                    consts2_sb[0:B, C_MASK:C_MASK + ROW].rearrange(
                        "p (a c) -> p a c", a=B
                    ),
                    po_sb[:, None, :].broadcast_to((B, B, CPC)),
                    mybir.AluOpType.mult,
                ).then_inc(s_o5, 1)
                vector.wait_ge(s_mm4, 1)
                nc.vector.tensor_copy(rep_sb[:, 0, :], prep[:]).then_inc(s_rep, 1)
                nc.vector.tensor_copy(rep_sb[:, 1, :], prep[:]).then_inc(s_rep, 1)
                nc.vector.tensor_copy(rep_sb[:, 2, :], prep[:]).then_inc(s_rep, 1)

    nc.compile()
    return nc


def _get_nc():
    if "nc" not in _CACHE:
        _CACHE["nc"] = _build_nc()
    return _CACHE["nc"]


def _prepare_in_maps(context, Wv, Wo, bo):
    context = np.ascontiguousarray(context, dtype=np.float32)
    Wv = np.ascontiguousarray(Wv, dtype=np.float32)
    Wo = np.ascontiguousarray(Wo, dtype=np.float32)
    bo = np.ascontiguousarray(bo, dtype=np.float32)

    # wvx[p, k, 0:4] = context[b, 128k+p]; wvx[p, k, 4:] = Wv[128k+p, :]
    wvx = np.empty((P, KC, B + DIM), dtype=BFNP)
    ctx_chunks = context.T.reshape(KC, P, B)          # [k, p, b]
    wv_chunks = Wv.reshape(KC, P, DIM)                # [k, p, d]
    wvx[:, :, 0:B] = ctx_chunks.transpose(1, 0, 2).astype(BFNP)
    wvx[:, :, B:] = wv_chunks.transpose(1, 0, 2).astype(BFNP)
    wvx = np.ascontiguousarray(wvx)

    # constw[p, m*64+c] = Wo[128m+p, 64i+c]
    wo_chunk = Wo.reshape(KD, P, DIM).transpose(1, 0, 2)  # [p, m, d_out]

    mask = np.zeros((B, B, CPC), dtype=BFNP)
    for b in range(B):
        mask[b, b, :] = 1.0

    id4 = np.eye(B, dtype=np.float32).view(BFNP)      # [4, 8] bf16 bit view

    in_maps = []
    for i in range(NCORES):
        constw = np.ascontiguousarray(
            wo_chunk[:, :, i * CPC:(i + 1) * CPC].reshape(P, KD * CPC).astype(BFNP)
        )
        consts2 = np.zeros((36, C2W), dtype=BFNP)
        consts2[0:5, C_ONES:C_ONES + P] = 1.0
        consts2[0:B, C_MASK:C_MASK + ROW] = mask.reshape(B, ROW)
        consts2[4, C_O5:C_O5 + ROW] = np.tile(
            bo[i * CPC:(i + 1) * CPC], B
        ).astype(BFNP)
        consts2[0:B, C_ID:C_ID + 8] = id4
        consts2[32:36, C_ID:C_ID + 8] = id4
        in_maps.append(
            {
                "wvx": wvx,
                "constw": constw,
                "consts2": np.ascontiguousarray(consts2),
                "idc": np.eye(B, dtype=np.float32),
            }
        )
    return in_maps


def _unshard(results):
    shards = np.stack([r["outd"] for r in results], axis=0)
    shards = shards.reshape(NCORES, NPOS, B, CPC)
    out = shards.transpose(2, 0, 3, 1).reshape(B, DIM, H, W)
    return np.ascontiguousarray(out)


def kernel(x, context, Wq, Wk, Wv, Wo, bo):
    del x, Wq, Wk
    nc = _get_nc()
    in_maps = _prepare_in_maps(context, Wv, Wo, bo)
    results = run_bass_kernel_spmd(nc, in_maps, list(range(NCORES))).results
    return _unshard(results)


# revision 16
# speedup vs baseline: 1.1550x; 1.0605x over previous
"""Trainium2 Bass kernel for nn_CrossAttention_15006615733765 (raw Bass, no Tile).

Mathematical structure: the reference broadcasts a per-batch context vector
(B, CTX_DIM) to every spatial position before projecting to K/V.  All keys
within a batch are therefore identical, softmax over the key axis is exactly
uniform, and the attention output equals V itself.  The module collapses to

    out[b, c, h, w] = ((context[b] @ Wv) @ Wo + bo)[c]

independent of x, Wq and Wk.  The kernel computes the two small matmuls on
the tensor engine in bf16 (tolerance 2e-2 >> bf16 rounding ~1e-3) and
materializes the broadcast output shard per core, sharding the 512 output
channels across 8 cores.

v2 design (from trace analysis of the f32 v1 at 23.4us):
  - measured exec window ends ~7us after the kernel body exits (walrus
    resets all 254 semaphores, Tensor's 52 resets at ~115ns are the long
    pole), concurrently with the 2.36MB output store (~7us at ~337GB/s).
    Both tails are ~equal, so exec ~= y_ready + 2.7us.  Everything below
    minimizes y_ready.
  - bf16 weights halve the input DMA (1.65MB -> 0.86MB) and halve the PE
    passes (no fp32 LOW/HIGH double pumping).
  - ctx chunks are packed WITH the Wv chunks (wvx[:, k, 0:4] = ctx chunk)
    so one dma_start per chunk-group feeds both operands; DMA_DIRECT2D
    issue costs ~0.65us each, so chunk DMAs are batched 1+2 per queue.
  - stage1 is 2-way column-tiled on the PE (tile (0,0) streams wv cols
    0:256 into psum rows 0:4, tile (0,32) streams cols 256:512 into psum
    rows 32:36) so the two halves run concurrently in the array.
  - the final broadcast is ONE matmul (ones [5,128] stationary x block-diag
    o5big [5,256]) instead of 4 selector matmuls; the block-diag is built
    by a single DVE masked multiply from po.
  - bias enters as a host-prepared row of the const tensor (o5big row 4).
  - the output store is split across both HWDGE queues (sync/scalar).
Engine plan:
  Sync   : wvx chunks 0,1-2; output store half A
  Scalar : wvx chunks 3,4-5; rep replica 1 copy; output store half B
  GpSimd : consts (SWDGE), otherwise idle (block exits no_gpsimd_drain)
  Tensor : warmup -> stage1 (col-tiled) -> transposes -> stage2 -> bcast
  Vector : PSUM->SBUF copies, masked multiply, rep replicas 0/2
"""

import numpy as np
import ml_dtypes

import concourse.bacc as bacc
import concourse.mybir as mybir
from concourse.bass_utils import run_bass_kernel_spmd

B, DIM, CTX_DIM = 4, 512, 768
H = W = 48
NPOS = H * W
NCORES = 8
CPC = DIM // NCORES          # 64 channels per core
P = 128
KC = CTX_DIM // P            # 6 contraction chunks
KD = DIM // P                # 4 d-chunks
ROW = B * CPC                # 256 floats per output row
NDUP = 2                     # row duplication -> 2 KiB store descriptors
F32 = mybir.dt.float32
BF16 = mybir.dt.bfloat16
BFNP = ml_dtypes.bfloat16

# consts2 [5, 648] column layout
C_ONES = 0        # [5, 128]  all-ones selector (stationary of bcast matmul)
C_MASK = 128      # [4, 256]  block-diag mask
C_O5 = 384        # [5, 256]  o5big: rows 0-3 runtime (masked y), row 4 bias
C_ID = 640        # [4, 8]    f32 4x4 identity, stored as bf16 bit pattern
C2W = 648

_CACHE: dict = {}
NWARM = 3  # ungated PE warmup matmuls (0 for CoreSim: it rejects
           # reads of uninitialized SBUF)
COLTILE = True  # 2-way PE column tiling for stage1


def _build_nc():
    nc = bacc.Bacc("TRN2", target_bir_lowering=False, debug=False, num_devices=NCORES)

    wvx = nc.dram_tensor("wvx", [P, KC, B + DIM], BF16, kind="ExternalInput")
    constw = nc.dram_tensor("constw", [P, KD * CPC], BF16, kind="ExternalInput")
    consts2 = nc.dram_tensor("consts2", [36, C2W], BF16, kind="ExternalInput")
    idc = nc.dram_tensor("idc", [B, B], F32, kind="ExternalInput")
    outd = nc.dram_tensor("outd", [NPOS, ROW], F32, kind="ExternalOutput")

    wvx_sb = nc.alloc_sbuf_tensor("wvx_sb", [P, KC, B + DIM], BF16).ap()
    constw_sb = nc.alloc_sbuf_tensor("constw_sb", [P, KD * CPC], BF16).ap()
    consts2_sb = nc.alloc_sbuf_tensor("consts2_sb", [36, C2W], BF16).ap()
    idc_sb = nc.alloc_sbuf_tensor("idc_sb", [36, B], F32).ap()
    t_sb = nc.alloc_sbuf_tensor("t_sb", [36, DIM // 2], F32).ap()
    tf_sb = nc.alloc_sbuf_tensor("tf_sb", [B, DIM], F32).ap()
    tT_sb = nc.alloc_sbuf_tensor("tT_sb", [P, KD, B], BF16).ap()
    po_sb = nc.alloc_sbuf_tensor("po_sb", [B, CPC], BF16).ap()
    rep_sb = nc.alloc_sbuf_tensor("rep_sb", [P, NDUP, ROW], F32).ap()

    pt_a = nc.alloc_psum_tensor("pt_a", [B, DIM // 2], F32).ap()
    pt_b = nc.alloc_psum_tensor("pt_b", [36, DIM // 2], F32).ap()
    pwide = nc.alloc_psum_tensor("pwide", [B, DIM], F32).ap()
    ptT = nc.alloc_psum_tensor("ptT", [P, KD, B], F32).ap()
    po = nc.alloc_psum_tensor("po", [B, CPC], F32).ap()
    prep = nc.alloc_psum_tensor("prep", [P, ROW], F32).ap()
    pwarm = nc.alloc_psum_tensor("pwarm", [B, DIM], F32).ap()

    id_f32 = idc_sb[0:B, :]
    id_f32_hi = idc_sb[32:36, :]

    from contextlib import ExitStack

    with ExitStack() as stack:
        s_w = [stack.enter_context(nc.semaphore(f"s_w{k}")) for k in range(KC)]
        s_id = stack.enter_context(nc.semaphore("s_id"))
        s_c2 = stack.enter_context(nc.semaphore("s_c2"))
        s_cw = stack.enter_context(nc.semaphore("s_cw"))
        s_mm1 = stack.enter_context(nc.semaphore("s_mm1"))
        s_tcp = stack.enter_context(nc.semaphore("s_tcp"))
        s_tcb = stack.enter_context(nc.semaphore("s_tcb"))
        s_mm2 = stack.enter_context(nc.semaphore("s_mm2"))
        s_tTcp = stack.enter_context(nc.semaphore("s_tTcp"))
        s_mm3 = stack.enter_context(nc.semaphore("s_mm3"))
        s_o5 = stack.enter_context(nc.semaphore("s_o5"))
        s_po = stack.enter_context(nc.semaphore("s_po"))
        s_mm4 = stack.enter_context(nc.semaphore("s_mm4"))
        s_rep = stack.enter_context(nc.semaphore("s_rep"))
        s_out = stack.enter_context(nc.semaphore("s_out"))

        out_view = outd.rearrange("(r p d) n -> p r (d n)", p=P, d=NDUP)
        src_view = (
            rep_sb.rearrange("p d n -> p (d n)")[:, None, :]
            .broadcast_to((P, NPOS // (NDUP * P), NDUP * ROW))
        )
        RHALF = NPOS // (NDUP * P) // 2  # 3

        with nc.Block(no_gpsimd_drain=True) as block:

            @block.sync
            def _(sync):
                sync.dma_start(
                    out=wvx_sb[:, 0, :], in_=wvx[:, 0, :]
                ).then_inc(s_w[0], 16)
                sync.dma_start(
                    out=wvx_sb[:, 2, :], in_=wvx[:, 2, :]
                ).then_inc(s_w[2], 16)
                sync.dma_start(
                    out=wvx_sb[:, 4, :], in_=wvx[:, 4, :]
                ).then_inc(s_w[4], 16)
                sync.dma_start(
                    out=idc_sb[0:B, :], in_=idc[:]
                ).then_inc(s_id, 16)
                sync.dma_start(
                    out=idc_sb[32:36, :], in_=idc[:]
                ).then_inc(s_id, 16)
                sync.wait_ge(s_rep, 2)
                sync.dma_start(
                    out=out_view[:, 0:RHALF, :], in_=src_view[:, 0:RHALF, :]
                ).then_inc(s_out, 16)

            @block.scalar
            def _(scalar):
                scalar.dma_start(
                    out=wvx_sb[:, 1, :], in_=wvx[:, 1, :]
                ).then_inc(s_w[1], 16)
                scalar.dma_start(
                    out=wvx_sb[:, 3, :], in_=wvx[:, 3, :]
                ).then_inc(s_w[3], 16)
                scalar.dma_start(
                    out=wvx_sb[:, 5, :], in_=wvx[:, 5, :]
                ).then_inc(s_w[5], 16)
                scalar.dma_start(out=consts2_sb[:], in_=consts2[:]).then_inc(
                    s_c2, 16
                )
                scalar.dma_start(out=constw_sb[:], in_=constw[:]).then_inc(
                    s_cw, 16
                )
                scalar.wait_ge(s_rep, 2)
                scalar.dma_start(
                    out=out_view[:, RHALF:, :], in_=src_view[:, RHALF:, :]
                ).then_inc(s_out, 16)

            @block.gpsimd
            def _(gpsimd):
                pass

            @block.tensor
            def _(tensor):
                # Ungated warmup matmuls on garbage SBUF ramp the PE clock
                # (1.2 -> 2.4 GHz needs ~4-5us of sustained activity).
                for w in range(NWARM):
                    nc.tensor.matmul(
                        pwarm[:],
                        wvx_sb[:, 0, 0:B],
                        wvx_sb[:, KC - 1, B:],
                        start=(w == 0),
                        stop=(w == NWARM - 1),
                    )

                # stage1: t[b, d] = sum_c ctx[b, c] Wv[c, d]
                HN = DIM // 2
                order = [(k, s_w[k]) for k in range(KC)]
                seen = 0
                # Tile A (array cols 0-3) streams wv cols 0:256 ->
                # pt_a[0:4], tile B (cols 32-35) streams cols 256:512
                # -> pt_b[32:36]; the two halves run concurrently.
                last_a = last_b = None
                for k, sem in order:
                    if sem is not None:
                        tensor.wait_ge(sem, 16)
                    last_a = nc.tensor.matmul(
                        pt_a[:],
                        wvx_sb[:, k, 0:B],
                        wvx_sb[:, k, B:B + HN],
                        start=(seen == 0),
                        stop=(seen == KC - 1),
                        tile_position=(0, 0),
                        skip_group_check=True,
                    )
                    last_b = nc.tensor.matmul(
                        pt_b[32:32 + B, :],
                        wvx_sb[:, k, 0:B],
                        wvx_sb[:, k, B + HN:],
                        start=(seen == 0),
                        stop=(seen == KC - 1),
                        tile_position=(0, 32),
                        skip_group_check=True,
                    )
                    seen += 1
                last_a.then_inc(s_mm1, 1)
                last_b.then_inc(s_mm1, 1)

                # transposes: tT[d, b] per 128-d chunk (f32, identity mult)
                tensor.wait_ge(s_id, 32)
                tensor.wait_ge(s_tcp, 1)
                nc.tensor.transpose(ptT[:, 0, :], t_sb[0:B, 0:P], id_f32)
                ins = nc.tensor.transpose(
                    ptT[:, 1, :], t_sb[0:B, P:2 * P], id_f32
                )
                tensor.wait_ge(s_tcb, 1)
                nc.tensor.transpose(
                    ptT[:, 2, :], t_sb[32:32 + B, 0:P], id_f32_hi
                )
                ins = nc.tensor.transpose(
                    ptT[:, 3, :], t_sb[32:32 + B, P:2 * P], id_f32_hi
                )
                ins.then_inc(s_mm2, 1)

                # stage2: po[b, c] = sum_d tT[d, b] Wo[d, c]
                tensor.wait_ge(s_cw, 16)
                tensor.wait_ge(s_tTcp, 1)
                for m in range(KD):
                    ins = nc.tensor.matmul(
                        po[:],
                        tT_sb[:, m, :],
                        constw_sb[:, m * CPC:(m + 1) * CPC],
                        start=(m == 0),
                        stop=(m == KD - 1),
                    )
                ins.then_inc(s_mm3, 1)

                # broadcast: prep[p, n] = sum_k ones[k] * o5big[k, n]
                #          = y[b(n), c(n)] + bo[c(n)]  on every partition
                tensor.wait_ge(s_o5, 1)
                ins = nc.tensor.matmul(
                    prep[:],
                    consts2_sb[0:5, C_ONES:C_ONES + P],
                    consts2_sb[0:5, C_O5:C_O5 + ROW],
                    start=True,
                    stop=True,
                )
                ins.then_inc(s_mm4, 1)

            @block.vector
            def _(vector):
                HN = DIM // 2
                vector.wait_ge(s_mm1, 2)
                nc.vector.tensor_copy(t_sb[0:B, :], pt_a[:]).then_inc(s_tcp, 1)
                nc.vector.tensor_copy(
                    t_sb[32:32 + B, :], pt_b[32:32 + B, :]
                ).then_inc(s_tcb, 1)
                vector.wait_ge(s_mm2, 1)
                nc.vector.tensor_copy(tT_sb[:], ptT[:]).then_inc(s_tTcp, 1)
                # masked multiply builds the block-diag o5big rows 0-3
                vector.wait_ge(s_mm3, 1)
                vector.wait_ge(s_c2, 16)
                nc.vector.tensor_copy(po_sb[:], po[:]).then_inc(s_po, 1)
                vector.wait_ge(s_po, 1)
                nc.vector.tensor_tensor(
                    consts2_sb[0:B, C_O5:C_O5 + ROW].rearrange(
                        "p (a c) -> p a c", a=B
                    ),
                    consts2_sb[0:B, C_MASK:C_MASK + ROW].rearrange(
                        "p (a c) -> p a c", a=B
                    ),
                    po_sb[:, None, :].broadcast_to((B, B, CPC)),
                    mybir.AluOpType.mult,
                ).then_inc(s_o5, 1)
                vector.wait_ge(s_mm4, 1)
                nc.vector.tensor_copy(rep_sb[:, 0, :], prep[:]).then_inc(s_rep, 1)
                nc.vector.tensor_copy(rep_sb[:, 1, :], prep[:]).then_inc(s_rep, 1)

    nc.compile()
    return nc


def _get_nc():
    if "nc" not in _CACHE:
        _CACHE["nc"] = _build_nc()
    return _CACHE["nc"]


def _prepare_in_maps(context, Wv, Wo, bo):
    context = np.ascontiguousarray(context, dtype=np.float32)
    Wv = np.ascontiguousarray(Wv, dtype=np.float32)
    Wo = np.ascontiguousarray(Wo, dtype=np.float32)
    bo = np.ascontiguousarray(bo, dtype=np.float32)

    # wvx[p, k, 0:4] = context[b, 128k+p]; wvx[p, k, 4:] = Wv[128k+p, :]
    wvx = np.empty((P, KC, B + DIM), dtype=BFNP)
    ctx_chunks = context.T.reshape(KC, P, B)          # [k, p, b]
    wv_chunks = Wv.reshape(KC, P, DIM)                # [k, p, d]
    wvx[:, :, 0:B] = ctx_chunks.transpose(1, 0, 2).astype(BFNP)
    wvx[:, :, B:] = wv_chunks.transpose(1, 0, 2).astype(BFNP)
    wvx = np.ascontiguousarray(wvx)

    # constw[p, m*64+c] = Wo[128m+p, 64i+c]
    wo_chunk = Wo.reshape(KD, P, DIM).transpose(1, 0, 2)  # [p, m, d_out]

    mask = np.zeros((B, B, CPC), dtype=BFNP)
    for b in range(B):
        mask[b, b, :] = 1.0

    id4 = np.eye(B, dtype=np.float32).view(BFNP)      # [4, 8] bf16 bit view

    in_maps = []
    for i in range(NCORES):
        constw = np.ascontiguousarray(
            wo_chunk[:, :, i * CPC:(i + 1) * CPC].reshape(P, KD * CPC).astype(BFNP)
        )
        consts2 = np.zeros((36, C2W), dtype=BFNP)
        consts2[0:5, C_ONES:C_ONES + P] = 1.0
        consts2[0:B, C_MASK:C_MASK + ROW] = mask.reshape(B, ROW)
        consts2[4, C_O5:C_O5 + ROW] = np.tile(
            bo[i * CPC:(i + 1) * CPC], B
        ).astype(BFNP)
        consts2[0:B, C_ID:C_ID + 8] = id4
        consts2[32:36, C_ID:C_ID + 8] = id4
        in_maps.append(
            {
                "wvx": wvx,
                "constw": constw,
                "consts2": np.ascontiguousarray(consts2),
                "idc": np.eye(B, dtype=np.float32),
            }
        )
    return in_maps


def _unshard(results):
    shards = np.stack([r["outd"] for r in results], axis=0)
    shards = shards.reshape(NCORES, NPOS, B, CPC)
    out = shards.transpose(2, 0, 3, 1).reshape(B, DIM, H, W)
    return np.ascontiguousarray(out)


def kernel(x, context, Wq, Wk, Wv, Wo, bo):
    del x, Wq, Wk
    nc = _get_nc()
    in_maps = _prepare_in_maps(context, Wv, Wo, bo)
    results = run_bass_kernel_spmd(nc, in_maps, list(range(NCORES))).results
    return _unshard(results)


# revision 17
# speedup vs baseline: 1.2088x; 1.0465x over previous
"""Trainium2 Bass kernel for nn_CrossAttention_15006615733765 (raw Bass, no Tile).

Mathematical structure: the reference broadcasts a per-batch context vector
(B, CTX_DIM) to every spatial position before projecting to K/V.  All keys
within a batch are therefore identical, softmax over the key axis is exactly
uniform, and the attention output equals V itself.  The module collapses to

    out[b, c, h, w] = ((context[b] @ Wv) @ Wo + bo)[c]

independent of x, Wq and Wk.  The kernel computes the two small matmuls on
the tensor engine in bf16 (tolerance 2e-2 >> bf16 rounding ~1e-3) and
materializes the broadcast output shard per core, sharding the 512 output
channels across 8 cores.

v2 design (from trace analysis of the f32 v1 at 23.4us):
  - measured exec window ends ~7us after the kernel body exits (walrus
    resets all 254 semaphores, Tensor's 52 resets at ~115ns are the long
    pole), concurrently with the 2.36MB output store (~7us at ~337GB/s).
    Both tails are ~equal, so exec ~= y_ready + 2.7us.  Everything below
    minimizes y_ready.
  - bf16 weights halve the input DMA (1.65MB -> 0.86MB) and halve the PE
    passes (no fp32 LOW/HIGH double pumping).
  - ctx chunks are packed WITH the Wv chunks (wvx[:, k, 0:4] = ctx chunk)
    so one dma_start per chunk-group feeds both operands; DMA_DIRECT2D
    issue costs ~0.65us each, so chunk DMAs are batched 1+2 per queue.
  - stage1 is 2-way column-tiled on the PE (tile (0,0) streams wv cols
    0:256 into psum rows 0:4, tile (0,32) streams cols 256:512 into psum
    rows 32:36) so the two halves run concurrently in the array.
  - the final broadcast is ONE matmul (ones [5,128] stationary x block-diag
    o5big [5,256]) instead of 4 selector matmuls; the block-diag is built
    by a single DVE masked multiply from po.
  - bias enters as a host-prepared row of the const tensor (o5big row 4).
  - the output store is split across both HWDGE queues (sync/scalar).
Engine plan:
  Sync   : wvx chunks 0,1-2; output store half A
  Scalar : wvx chunks 3,4-5; rep replica 1 copy; output store half B
  GpSimd : consts (SWDGE), otherwise idle (block exits no_gpsimd_drain)
  Tensor : warmup -> stage1 (col-tiled) -> transposes -> stage2 -> bcast
  Vector : PSUM->SBUF copies, masked multiply, rep replicas 0/2
"""

import numpy as np
import ml_dtypes

import concourse.bacc as bacc
import concourse.mybir as mybir
from concourse.bass_utils import run_bass_kernel_spmd

B, DIM, CTX_DIM = 4, 512, 768
H = W = 48
NPOS = H * W
NCORES = 8
CPC = DIM // NCORES          # 64 channels per core
P = 128
KC = CTX_DIM // P            # 6 contraction chunks
KD = DIM // P                # 4 d-chunks
ROW = B * CPC                # 256 floats per output row
NDUP = 2                     # row duplication -> 2 KiB store descriptors
F32 = mybir.dt.float32
BF16 = mybir.dt.bfloat16
BFNP = ml_dtypes.bfloat16

# consts2 [5, 648] column layout
C_ONES = 0        # [5, 128]  all-ones selector (stationary of bcast matmul)
C_MASK = 128      # [4, 256]  block-diag mask
C_O5 = 384        # [5, 256]  o5big: rows 0-3 runtime (masked y), row 4 bias
C_ID = 640        # [4, 8]    f32 4x4 identity, stored as bf16 bit pattern
C2W = 648

_CACHE: dict = {}
NWARM = 5  # ungated PE warmup matmuls (0 for CoreSim: it rejects
           # reads of uninitialized SBUF)
COLTILE = True  # 2-way PE column tiling for stage1


def _build_nc():
    nc = bacc.Bacc("TRN2", target_bir_lowering=False, debug=False, num_devices=NCORES)

    wvx = nc.dram_tensor("wvx", [P, KC, B + DIM], BF16, kind="ExternalInput")
    constw = nc.dram_tensor("constw", [P, KD * CPC], BF16, kind="ExternalInput")
    consts2 = nc.dram_tensor("consts2", [36, C2W], BF16, kind="ExternalInput")
    idc = nc.dram_tensor("idc", [36, B], F32, kind="ExternalInput")
    outd = nc.dram_tensor("outd", [NPOS, ROW], F32, kind="ExternalOutput")

    wvx_sb = nc.alloc_sbuf_tensor("wvx_sb", [P, KC, B + DIM], BF16).ap()
    constw_sb = nc.alloc_sbuf_tensor("constw_sb", [P, KD * CPC], BF16).ap()
    consts2_sb = nc.alloc_sbuf_tensor("consts2_sb", [36, C2W], BF16).ap()
    idc_sb = nc.alloc_sbuf_tensor("idc_sb", [36, B], F32).ap()
    t_sb = nc.alloc_sbuf_tensor("t_sb", [36, DIM // 2], F32).ap()
    tf_sb = nc.alloc_sbuf_tensor("tf_sb", [B, DIM], F32).ap()
    tT_sb = nc.alloc_sbuf_tensor("tT_sb", [P, KD, B], BF16).ap()
    po_sb = nc.alloc_sbuf_tensor("po_sb", [B, CPC], BF16).ap()
    rep_sb = nc.alloc_sbuf_tensor("rep_sb", [P, NDUP, ROW], F32).ap()

    pt_a = nc.alloc_psum_tensor("pt_a", [B, DIM // 2], F32).ap()
    pt_b = nc.alloc_psum_tensor("pt_b", [36, DIM // 2], F32).ap()
    pwide = nc.alloc_psum_tensor("pwide", [B, DIM], F32).ap()
    ptT = nc.alloc_psum_tensor("ptT", [P, KD, B], F32).ap()
    po = nc.alloc_psum_tensor("po", [B, CPC], F32).ap()
    prep = nc.alloc_psum_tensor("prep", [P, ROW], F32).ap()
    pwarm = nc.alloc_psum_tensor("pwarm", [B, DIM], F32).ap()

    id_f32 = idc_sb[0:B, :]
    id_f32_hi = idc_sb[32:36, :]

    from contextlib import ExitStack

    with ExitStack() as stack:
        s_w01 = stack.enter_context(nc.semaphore("s_w01"))
        s_w23 = stack.enter_context(nc.semaphore("s_w23"))
        s_w45 = stack.enter_context(nc.semaphore("s_w45"))
        s_id = stack.enter_context(nc.semaphore("s_id"))
        s_c2 = stack.enter_context(nc.semaphore("s_c2"))
        s_cw = stack.enter_context(nc.semaphore("s_cw"))
        s_mm1 = stack.enter_context(nc.semaphore("s_mm1"))
        s_tcp = stack.enter_context(nc.semaphore("s_tcp"))
        s_tcb = stack.enter_context(nc.semaphore("s_tcb"))
        s_mm2 = stack.enter_context(nc.semaphore("s_mm2"))
        s_tTcp = stack.enter_context(nc.semaphore("s_tTcp"))
        s_mm3 = stack.enter_context(nc.semaphore("s_mm3"))
        s_o5 = stack.enter_context(nc.semaphore("s_o5"))
        s_po = stack.enter_context(nc.semaphore("s_po"))
        s_mm4 = stack.enter_context(nc.semaphore("s_mm4"))
        s_rep = stack.enter_context(nc.semaphore("s_rep"))
        s_out = stack.enter_context(nc.semaphore("s_out"))

        out_view = outd.rearrange("(r p d) n -> p r (d n)", p=P, d=NDUP)
        src_view = (
            rep_sb.rearrange("p d n -> p (d n)")[:, None, :]
            .broadcast_to((P, NPOS // (NDUP * P), NDUP * ROW))
        )
        RHALF = NPOS // (NDUP * P) // 2  # 3

        with nc.Block(no_gpsimd_drain=True) as block:

            @block.sync
            def _(sync):
                sync.dma_start(
                    out=wvx_sb[:, 0:2, :], in_=wvx[:, 0:2, :]
                ).then_inc(s_w01, 16)
                sync.dma_start(out=constw_sb[:], in_=constw[:]).then_inc(
                    s_cw, 16
                )
                sync.wait_ge(s_rep, 2)
                sync.dma_start(
                    out=out_view[:, 0:RHALF, :], in_=src_view[:, 0:RHALF, :]
                ).then_inc(s_out, 16)

            @block.scalar
            def _(scalar):
                scalar.dma_start(
                    out=wvx_sb[:, 2:4, :], in_=wvx[:, 2:4, :]
                ).then_inc(s_w23, 16)
                scalar.dma_start(out=idc_sb[:], in_=idc[:]).then_inc(s_id, 16)
                scalar.dma_start(out=consts2_sb[:], in_=consts2[:]).then_inc(
                    s_c2, 16
                )
                scalar.wait_ge(s_rep, 2)
                scalar.dma_start(
                    out=out_view[:, RHALF:, :], in_=src_view[:, RHALF:, :]
                ).then_inc(s_out, 16)

            @block.gpsimd
            def _(gpsimd):
                gpsimd.dma_start(
                    out=wvx_sb[:, 4:6, :], in_=wvx[:, 4:6, :]
                ).then_inc(s_w45, 16)

            @block.tensor
            def _(tensor):
                # Ungated warmup matmuls on garbage SBUF ramp the PE clock
                # (1.2 -> 2.4 GHz needs ~4-5us of sustained activity).
                for w in range(NWARM):
                    nc.tensor.matmul(
                        pwarm[:],
                        wvx_sb[:, 0, 0:B],
                        wvx_sb[:, KC - 1, B:],
                        start=(w == 0),
                        stop=(w == NWARM - 1),
                    )

                # stage1: t[b, d] = sum_c ctx[b, c] Wv[c, d]
                HN = DIM // 2
                order = [(0, s_w01), (1, None), (2, s_w23), (3, None),
                         (4, s_w45), (5, None)]
                seen = 0
                # Tile A (array cols 0-3) streams wv cols 0:256 ->
                # pt_a[0:4], tile B (cols 32-35) streams cols 256:512
                # -> pt_b[32:36]; the two halves run concurrently.
                last_a = last_b = None
                for k, sem in order:
                    if sem is not None:
                        tensor.wait_ge(sem, 16)
                    last_a = nc.tensor.matmul(
                        pt_a[:],
                        wvx_sb[:, k, 0:B],
                        wvx_sb[:, k, B:B + HN],
                        start=(seen == 0),
                        stop=(seen == KC - 1),
                        tile_position=(0, 0),
                        skip_group_check=True,
                    )
                    last_b = nc.tensor.matmul(
                        pt_b[32:32 + B, :],
                        wvx_sb[:, k, 0:B],
                        wvx_sb[:, k, B + HN:],
                        start=(seen == 0),
                        stop=(seen == KC - 1),
                        tile_position=(0, 32),
                        skip_group_check=True,
                    )
                    seen += 1
                last_a.then_inc(s_mm1, 1)
                last_b.then_inc(s_mm1, 1)

                # transposes: tT[d, b] per 128-d chunk (f32, identity mult)
                tensor.wait_ge(s_id, 16)
                tensor.wait_ge(s_tcp, 1)
                nc.tensor.transpose(ptT[:, 0, :], t_sb[0:B, 0:P], id_f32)
                ins = nc.tensor.transpose(
                    ptT[:, 1, :], t_sb[0:B, P:2 * P], id_f32
                )
                tensor.wait_ge(s_tcb, 1)
                nc.tensor.transpose(
                    ptT[:, 2, :], t_sb[32:32 + B, 0:P], id_f32_hi
                )
                ins = nc.tensor.transpose(
                    ptT[:, 3, :], t_sb[32:32 + B, P:2 * P], id_f32_hi
                )
                ins.then_inc(s_mm2, 1)

                # stage2: po[b, c] = sum_d tT[d, b] Wo[d, c]
                tensor.wait_ge(s_cw, 16)
                tensor.wait_ge(s_tTcp, 1)
                for m in range(KD):
                    ins = nc.tensor.matmul(
                        po[:],
                        tT_sb[:, m, :],
                        constw_sb[:, m * CPC:(m + 1) * CPC],
                        start=(m == 0),
                        stop=(m == KD - 1),
                    )
                ins.then_inc(s_mm3, 1)

                # broadcast: prep[p, n] = sum_k ones[k] * o5big[k, n]
                #          = y[b(n), c(n)] + bo[c(n)]  on every partition
                tensor.wait_ge(s_o5, 1)
                ins = nc.tensor.matmul(
                    prep[:],
                    consts2_sb[0:5, C_ONES:C_ONES + P],
                    consts2_sb[0:5, C_O5:C_O5 + ROW],
                    start=True,
                    stop=True,
                )
                ins.then_inc(s_mm4, 1)

            @block.vector
            def _(vector):
                HN = DIM // 2
                vector.wait_ge(s_mm1, 2)
                nc.vector.tensor_copy(t_sb[0:B, :], pt_a[:]).then_inc(s_tcp, 1)
                nc.vector.tensor_copy(
                    t_sb[32:32 + B, :], pt_b[32:32 + B, :]
                ).then_inc(s_tcb, 1)
                vector.wait_ge(s_mm2, 1)
                nc.vector.tensor_copy(tT_sb[:], ptT[:]).then_inc(s_tTcp, 1)
                # masked multiply builds the block-diag o5big rows 0-3
                vector.wait_ge(s_mm3, 1)
                vector.wait_ge(s_c2, 16)
                nc.vector.tensor_copy(po_sb[:], po[:]).then_inc(s_po, 1)
                vector.wait_ge(s_po, 1)
                nc.vector.tensor_tensor(
                    consts2_sb[0:B, C_O5:C_O5 + ROW].rearrange(
                        "p (a c) -> p a c", a=B
                    ),
                    consts2_sb[0:B, C_MASK:C_MASK + ROW].rearrange(
                        "p (a c) -> p a c", a=B
                    ),
                    po_sb[:, None, :].broadcast_to((B, B, CPC)),
                    mybir.AluOpType.mult,
                ).then_inc(s_o5, 1)
                vector.wait_ge(s_mm4, 1)
                nc.vector.tensor_copy(rep_sb[:, 0, :], prep[:]).then_inc(s_rep, 1)
                nc.vector.tensor_copy(rep_sb[:, 1, :], prep[:]).then_inc(s_rep, 1)

    nc.compile()
    return nc


def _get_nc():
    if "nc" not in _CACHE:
        _CACHE["nc"] = _build_nc()
    return _CACHE["nc"]


def _prepare_in_maps(context, Wv, Wo, bo):
    context = np.ascontiguousarray(context, dtype=np.float32)
    Wv = np.ascontiguousarray(Wv, dtype=np.float32)
    Wo = np.ascontiguousarray(Wo, dtype=np.float32)
    bo = np.ascontiguousarray(bo, dtype=np.float32)

    # wvx[p, k, 0:4] = context[b, 128k+p]; wvx[p, k, 4:] = Wv[128k+p, :]
    wvx = np.empty((P, KC, B + DIM), dtype=BFNP)
    ctx_chunks = context.T.reshape(KC, P, B)          # [k, p, b]
    wv_chunks = Wv.reshape(KC, P, DIM)                # [k, p, d]
    wvx[:, :, 0:B] = ctx_chunks.transpose(1, 0, 2).astype(BFNP)
    wvx[:, :, B:] = wv_chunks.transpose(1, 0, 2).astype(BFNP)
    wvx = np.ascontiguousarray(wvx)

    # constw[p, m*64+c] = Wo[128m+p, 64i+c]
    wo_chunk = Wo.reshape(KD, P, DIM).transpose(1, 0, 2)  # [p, m, d_out]

    mask = np.zeros((B, B, CPC), dtype=BFNP)
    for b in range(B):
        mask[b, b, :] = 1.0

    id4 = np.eye(B, dtype=np.float32).view(BFNP)      # [4, 8] bf16 bit view
    idc36 = np.zeros((36, B), dtype=np.float32)
    idc36[0:B] = np.eye(B, dtype=np.float32)
    idc36[32:36] = np.eye(B, dtype=np.float32)

    in_maps = []
    for i in range(NCORES):
        constw = np.ascontiguousarray(
            wo_chunk[:, :, i * CPC:(i + 1) * CPC].reshape(P, KD * CPC).astype(BFNP)
        )
        consts2 = np.zeros((36, C2W), dtype=BFNP)
        consts2[0:5, C_ONES:C_ONES + P] = 1.0
        consts2[0:B, C_MASK:C_MASK + ROW] = mask.reshape(B, ROW)
        consts2[4, C_O5:C_O5 + ROW] = np.tile(
            bo[i * CPC:(i + 1) * CPC], B
        ).astype(BFNP)
        consts2[0:B, C_ID:C_ID + 8] = id4
        consts2[32:36, C_ID:C_ID + 8] = id4
        in_maps.append(
            {
                "wvx": wvx,
                "constw": constw,
                "consts2": np.ascontiguousarray(consts2),
                "idc": idc36,
            }
        )
    return in_maps


def _unshard(results):
    shards = np.stack([r["outd"] for r in results], axis=0)
    shards = shards.reshape(NCORES, NPOS, B, CPC)
    out = shards.transpose(2, 0, 3, 1).reshape(B, DIM, H, W)
    return np.ascontiguousarray(out)


def kernel(x, context, Wq, Wk, Wv, Wo, bo):
    del x, Wq, Wk
    nc = _get_nc()
    in_maps = _prepare_in_maps(context, Wv, Wo, bo)
    results = run_bass_kernel_spmd(nc, in_maps, list(range(NCORES))).results
    return _unshard(results)


# revision 19
# speedup vs baseline: 1.2274x; 1.0154x over previous
"""Trainium2 Bass kernel for nn_CrossAttention_15006615733765 (raw Bass, no Tile).

Mathematical structure: the reference broadcasts a per-batch context vector
(B, CTX_DIM) to every spatial position before projecting to K/V.  All keys
within a batch are therefore identical, softmax over the key axis is exactly
uniform, and the attention output equals V itself.  The module collapses to

    out[b, c, h, w] = ((context[b] @ Wv) @ Wo + bo)[c]

independent of x, Wq and Wk.  The kernel computes the two small matmuls on
the tensor engine in bf16 (tolerance 2e-2 >> bf16 rounding ~1e-3) and
materializes the broadcast output shard per core, sharding the 512 output
channels across 8 cores.

v2 design (from trace analysis of the f32 v1 at 23.4us):
  - measured exec window ends ~7us after the kernel body exits (walrus
    resets all 254 semaphores, Tensor's 52 resets at ~115ns are the long
    pole), concurrently with the 2.36MB output store (~7us at ~337GB/s).
    Both tails are ~equal, so exec ~= y_ready + 2.7us.  Everything below
    minimizes y_ready.
  - bf16 weights halve the input DMA (1.65MB -> 0.86MB) and halve the PE
    passes (no fp32 LOW/HIGH double pumping).
  - ctx chunks are packed WITH the Wv chunks (wvx[:, k, 0:4] = ctx chunk)
    so one dma_start per chunk-group feeds both operands; DMA_DIRECT2D
    issue costs ~0.65us each, so chunk DMAs are batched 1+2 per queue.
  - stage1 is 2-way column-tiled on the PE (tile (0,0) streams wv cols
    0:256 into psum rows 0:4, tile (0,32) streams cols 256:512 into psum
    rows 32:36) so the two halves run concurrently in the array.
  - the final broadcast is ONE matmul (ones [5,128] stationary x block-diag
    o5big [5,256]) instead of 4 selector matmuls; the block-diag is built
    by a single DVE masked multiply from po.
  - bias enters as a host-prepared row of the const tensor (o5big row 4).
  - the output store is split across both HWDGE queues (sync/scalar).
Engine plan:
  Sync   : wvx chunks 0,1-2; output store half A
  Scalar : wvx chunks 3,4-5; rep replica 1 copy; output store half B
  GpSimd : consts (SWDGE), otherwise idle (block exits no_gpsimd_drain)
  Tensor : warmup -> stage1 (col-tiled) -> transposes -> stage2 -> bcast
  Vector : PSUM->SBUF copies, masked multiply, rep replicas 0/2
"""

import numpy as np
import ml_dtypes

import concourse.bacc as bacc
import concourse.mybir as mybir
from concourse.bass_utils import run_bass_kernel_spmd

B, DIM, CTX_DIM = 4, 512, 768
H = W = 48
NPOS = H * W
NCORES = 8
CPC = DIM // NCORES          # 64 channels per core
P = 128
KC = CTX_DIM // P            # 6 contraction chunks
KD = DIM // P                # 4 d-chunks
ROW = B * CPC                # 256 floats per output row
NDUP = 2                     # row duplication -> 2 KiB store descriptors
F32 = mybir.dt.float32
BF16 = mybir.dt.bfloat16
BFNP = ml_dtypes.bfloat16

# consts2 [5, 648] column layout
C_ONES = 0        # [5, 128]  all-ones selector (stationary of bcast matmul)
C_MASK = 128      # [4, 256]  block-diag mask
C_O5 = 384        # [5, 256]  o5big: rows 0-3 runtime (masked y), row 4 bias
C_ID = 640        # [4, 8]    f32 4x4 identity, stored as bf16 bit pattern
C2W = 648

_CACHE: dict = {}
NWARM = 5  # ungated PE warmup matmuls (0 for CoreSim: it rejects
           # reads of uninitialized SBUF)
COLTILE = True  # 2-way PE column tiling for stage1


def _build_nc():
    nc = bacc.Bacc("TRN2", target_bir_lowering=False, debug=False, num_devices=NCORES)

    wvx = nc.dram_tensor("wvx", [P, KC, B + DIM], BF16, kind="ExternalInput")
    constw = nc.dram_tensor("constw", [P, KD * CPC], BF16, kind="ExternalInput")
    consts2 = nc.dram_tensor("consts2", [36, C2W], BF16, kind="ExternalInput")
    idc = nc.dram_tensor("idc", [36, B], F32, kind="ExternalInput")
    outd = nc.dram_tensor("outd", [NPOS, ROW], F32, kind="ExternalOutput")

    wvx_sb = nc.alloc_sbuf_tensor("wvx_sb", [P, KC, B + DIM], BF16).ap()
    constw_sb = nc.alloc_sbuf_tensor("constw_sb", [P, KD * CPC], BF16).ap()
    consts2_sb = nc.alloc_sbuf_tensor("consts2_sb", [36, C2W], BF16).ap()
    idc_sb = nc.alloc_sbuf_tensor("idc_sb", [36, B], F32).ap()
    t_sb = nc.alloc_sbuf_tensor("t_sb", [36, DIM // 2], F32).ap()
    tf_sb = nc.alloc_sbuf_tensor("tf_sb", [B, DIM], F32).ap()
    tT_sb = nc.alloc_sbuf_tensor("tT_sb", [P, KD, B], BF16).ap()
    po_sb = nc.alloc_sbuf_tensor("po_sb", [B, CPC], BF16).ap()
    rep_sb = nc.alloc_sbuf_tensor("rep_sb", [P, NDUP, ROW], F32).ap()

    pt_a = nc.alloc_psum_tensor("pt_a", [B, DIM // 2], F32).ap()
    pt_b = nc.alloc_psum_tensor("pt_b", [36, DIM // 2], F32).ap()
    pwide = nc.alloc_psum_tensor("pwide", [B, DIM], F32).ap()
    ptT = nc.alloc_psum_tensor("ptT", [P, KD, B], F32).ap()
    po = nc.alloc_psum_tensor("po", [B, CPC], F32).ap()
    prep = nc.alloc_psum_tensor("prep", [P, ROW], F32).ap()
    pwarm = nc.alloc_psum_tensor("pwarm", [B, DIM], F32).ap()

    id_f32 = idc_sb[0:B, :]
    id_f32_hi = idc_sb[32:36, :]

    from contextlib import ExitStack

    with ExitStack() as stack:
        s_w01 = stack.enter_context(nc.semaphore("s_w01"))
        s_w23 = stack.enter_context(nc.semaphore("s_w23"))
        s_w45 = stack.enter_context(nc.semaphore("s_w45"))
        s_id = stack.enter_context(nc.semaphore("s_id"))
        s_c2 = stack.enter_context(nc.semaphore("s_c2"))
        s_cw = stack.enter_context(nc.semaphore("s_cw"))
        s_mm1 = stack.enter_context(nc.semaphore("s_mm1"))
        s_mmb = stack.enter_context(nc.semaphore("s_mmb"))
        s_tcp = stack.enter_context(nc.semaphore("s_tcp"))
        s_tcb = stack.enter_context(nc.semaphore("s_tcb"))
        s_mm2 = stack.enter_context(nc.semaphore("s_mm2"))
        s_tTcp = stack.enter_context(nc.semaphore("s_tTcp"))
        s_mm3 = stack.enter_context(nc.semaphore("s_mm3"))
        s_o5 = stack.enter_context(nc.semaphore("s_o5"))
        s_po = stack.enter_context(nc.semaphore("s_po"))
        s_mm4 = stack.enter_context(nc.semaphore("s_mm4"))
        s_rep = stack.enter_context(nc.semaphore("s_rep"))
        s_out = stack.enter_context(nc.semaphore("s_out"))

        out_view = outd.rearrange("(r p d) n -> p r (d n)", p=P, d=NDUP)
        src_view = (
            rep_sb.rearrange("p d n -> p (d n)")[:, None, :]
            .broadcast_to((P, NPOS // (NDUP * P), NDUP * ROW))
        )
        RHALF = NPOS // (NDUP * P) // 2  # 3

        with nc.Block(no_gpsimd_drain=True) as block:

            @block.sync
            def _(sync):
                sync.dma_start(
                    out=wvx_sb[:, 0:2, :], in_=wvx[:, 0:2, :]
                ).then_inc(s_w01, 16)
                sync.dma_start(out=constw_sb[:], in_=constw[:]).then_inc(
                    s_cw, 16
                )
                sync.wait_ge(s_rep, 2)
                sync.dma_start(
                    out=out_view[:, 0:RHALF, :], in_=src_view[:, 0:RHALF, :]
                ).then_inc(s_out, 16)

            @block.scalar
            def _(scalar):
                scalar.dma_start(
                    out=wvx_sb[:, 2:4, :], in_=wvx[:, 2:4, :]
                ).then_inc(s_w23, 16)
                scalar.dma_start(out=idc_sb[:], in_=idc[:]).then_inc(s_id, 16)
                scalar.dma_start(out=consts2_sb[:], in_=consts2[:]).then_inc(
                    s_c2, 16
                )
                scalar.wait_ge(s_rep, 2)
                scalar.dma_start(
                    out=out_view[:, RHALF:, :], in_=src_view[:, RHALF:, :]
                ).then_inc(s_out, 16)

            @block.gpsimd
            def _(gpsimd):
                gpsimd.dma_start(
                    out=wvx_sb[:, 4:6, :], in_=wvx[:, 4:6, :]
                ).then_inc(s_w45, 16)

            @block.tensor
            def _(tensor):
                # Ungated warmup matmuls on garbage SBUF ramp the PE clock
                # (1.2 -> 2.4 GHz needs ~4-5us of sustained activity).
                for w in range(NWARM):
                    nc.tensor.matmul(
                        pwarm[:],
                        wvx_sb[:, 0, 0:B],
                        wvx_sb[:, KC - 1, B:],
                        start=(w == 0),
                        stop=(w == NWARM - 1),
                    )

                # stage1: t[b, d] = sum_c ctx[b, c] Wv[c, d]
                HN = DIM // 2
                order = [(0, s_w01), (1, None), (2, s_w23), (3, None),
                         (4, s_w45), (5, None)]
                FILL = {2: 1, 4: 1} if NWARM else {}  # short dummies
                seen = 0
                # Tile A (array cols 0-3) streams wv cols 0:256 ->
                # pt_a[0:4], tile B (cols 32-35) streams cols 256:512
                # -> pt_b[32:36]; the two halves run concurrently.
                last_a = last_b = None
                for k, sem in order:
                    for _f in range(FILL.get(k, 0)):
                        nc.tensor.matmul(
                            pwarm[:, 0:P],
                            wvx_sb[:, 0, 0:B],
                            wvx_sb[:, KC - 1, B:B + P],
                            start=True,
                            stop=True,
                        )
                    if sem is not None:
                        tensor.wait_ge(sem, 16)
                    last_a = nc.tensor.matmul(
                        pt_a[:],
                        wvx_sb[:, k, 0:B],
                        wvx_sb[:, k, B:B + HN],
                        start=(seen == 0),
                        stop=(seen == KC - 1),
                        tile_position=(0, 0),
                        skip_group_check=True,
                    )
                    last_b = nc.tensor.matmul(
                        pt_b[32:32 + B, :],
                        wvx_sb[:, k, 0:B],
                        wvx_sb[:, k, B + HN:],
                        start=(seen == 0),
                        stop=(seen == KC - 1),
                        tile_position=(0, 32),
                        skip_group_check=True,
                    )
                    seen += 1
                last_a.then_inc(s_mm1, 1)
                last_b.then_inc(s_mmb, 1)

                # transposes: tT[d, b] per 128-d chunk (f32, identity mult)
                tensor.wait_ge(s_id, 16)
                tensor.wait_ge(s_tcp, 1)
                nc.tensor.transpose(ptT[:, 0, :], t_sb[0:B, 0:P], id_f32)
                ins = nc.tensor.transpose(
                    ptT[:, 1, :], t_sb[0:B, P:2 * P], id_f32
                )
                tensor.wait_ge(s_tcb, 1)
                nc.tensor.transpose(
                    ptT[:, 2, :], t_sb[32:32 + B, 0:P], id_f32_hi
                )
                ins = nc.tensor.transpose(
                    ptT[:, 3, :], t_sb[32:32 + B, P:2 * P], id_f32_hi
                )
                ins.then_inc(s_mm2, 1)

                # stage2: po[b, c] = sum_d tT[d, b] Wo[d, c]
                tensor.wait_ge(s_cw, 16)
                tensor.wait_ge(s_tTcp, 1)
                for m in range(KD):
                    ins = nc.tensor.matmul(
                        po[:],
                        tT_sb[:, m, :],
                        constw_sb[:, m * CPC:(m + 1) * CPC],
                        start=(m == 0),
                        stop=(m == KD - 1),
                    )
                ins.then_inc(s_mm3, 1)

                # broadcast: prep[p, n] = sum_k ones[k] * o5big[k, n]
                #          = y[b(n), c(n)] + bo[c(n)]  on every partition
                tensor.wait_ge(s_o5, 1)
                ins = nc.tensor.matmul(
                    prep[:],
                    consts2_sb[0:5, C_ONES:C_ONES + P],
                    consts2_sb[0:5, C_O5:C_O5 + ROW],
                    start=True,
                    stop=True,
                )
                ins.then_inc(s_mm4, 1)

            @block.vector
            def _(vector):
                HN = DIM // 2
                vector.wait_ge(s_mm1, 1)
                nc.vector.tensor_copy(t_sb[0:B, :], pt_a[:]).then_inc(s_tcp, 1)
                vector.wait_ge(s_mmb, 1)
                nc.vector.tensor_copy(
                    t_sb[32:32 + B, :], pt_b[32:32 + B, :]
                ).then_inc(s_tcb, 1)
                vector.wait_ge(s_mm2, 1)
                nc.vector.tensor_copy(tT_sb[:], ptT[:]).then_inc(s_tTcp, 1)
                # masked multiply builds the block-diag o5big rows 0-3
                vector.wait_ge(s_mm3, 1)
                vector.wait_ge(s_c2, 16)
                nc.vector.tensor_tensor(
                    consts2_sb[0:B, C_O5:C_O5 + ROW].rearrange(
                        "p (a c) -> p a c", a=B
                    ),
                    consts2_sb[0:B, C_MASK:C_MASK + ROW].rearrange(
                        "p (a c) -> p a c", a=B
                    ),
                    po[:, None, :].broadcast_to((B, B, CPC)),
                    mybir.AluOpType.mult,
                ).then_inc(s_o5, 1)
                vector.wait_ge(s_mm4, 1)
                nc.vector.tensor_copy(rep_sb[:, 0, :], prep[:]).then_inc(s_rep, 1)
                nc.vector.tensor_copy(rep_sb[:, 1, :], prep[:]).then_inc(s_rep, 1)

    nc.compile()
    return nc


def _get_nc():
    if "nc" not in _CACHE:
        _CACHE["nc"] = _build_nc()
    return _CACHE["nc"]


def _prepare_in_maps(context, Wv, Wo, bo):
    context = np.ascontiguousarray(context, dtype=np.float32)
    Wv = np.ascontiguousarray(Wv, dtype=np.float32)
    Wo = np.ascontiguousarray(Wo, dtype=np.float32)
    bo = np.ascontiguousarray(bo, dtype=np.float32)

    # wvx[p, k, 0:4] = context[b, 128k+p]; wvx[p, k, 4:] = Wv[128k+p, :]
    wvx = np.empty((P, KC, B + DIM), dtype=BFNP)
    ctx_chunks = context.T.reshape(KC, P, B)          # [k, p, b]
    wv_chunks = Wv.reshape(KC, P, DIM)                # [k, p, d]
    wvx[:, :, 0:B] = ctx_chunks.transpose(1, 0, 2).astype(BFNP)
    wvx[:, :, B:] = wv_chunks.transpose(1, 0, 2).astype(BFNP)
    wvx = np.ascontiguousarray(wvx)

    # constw[p, m*64+c] = Wo[128m+p, 64i+c]
    wo_chunk = Wo.reshape(KD, P, DIM).transpose(1, 0, 2)  # [p, m, d_out]

    mask = np.zeros((B, B, CPC), dtype=BFNP)
    for b in range(B):
        mask[b, b, :] = 1.0

    id4 = np.eye(B, dtype=np.float32).view(BFNP)      # [4, 8] bf16 bit view
    idc36 = np.zeros((36, B), dtype=np.float32)
    idc36[0:B] = np.eye(B, dtype=np.float32)
    idc36[32:36] = np.eye(B, dtype=np.float32)

    in_maps = []
    for i in range(NCORES):
        constw = np.ascontiguousarray(
            wo_chunk[:, :, i * CPC:(i + 1) * CPC].reshape(P, KD * CPC).astype(BFNP)
        )
        consts2 = np.zeros((36, C2W), dtype=BFNP)
        consts2[0:5, C_ONES:C_ONES + P] = 1.0
        consts2[0:B, C_MASK:C_MASK + ROW] = mask.reshape(B, ROW)
        consts2[4, C_O5:C_O5 + ROW] = np.tile(
            bo[i * CPC:(i + 1) * CPC], B
        ).astype(BFNP)
        consts2[0:B, C_ID:C_ID + 8] = id4
        consts2[32:36, C_ID:C_ID + 8] = id4
        in_maps.append(
            {
                "wvx": wvx,
                "constw": constw,
                "consts2": np.ascontiguousarray(consts2),
                "idc": idc36,
            }
        )
    return in_maps


def _unshard(results):
    shards = np.stack([r["outd"] for r in results], axis=0)
    shards = shards.reshape(NCORES, NPOS, B, CPC)
    out = shards.transpose(2, 0, 3, 1).reshape(B, DIM, H, W)
    return np.ascontiguousarray(out)


def kernel(x, context, Wq, Wk, Wv, Wo, bo):
    del x, Wq, Wk
    nc = _get_nc()
    in_maps = _prepare_in_maps(context, Wv, Wo, bo)
    results = run_bass_kernel_spmd(nc, in_maps, list(range(NCORES))).results
    return _unshard(results)
